# revision 16
# baseline (speedup 1.0000x reference)
"""NVFP4 (E2M1, block-16) dequant matmul on 8 TRN2 NeuronCores — v8.

out[m, n] = sum_k (LUT[x[m,k]] * xs[m,k//16] * gx) * (LUT[w[n,k]] * ws[n,k//16] * gw) + bias[n]

Sharding: tensor-parallel along N: each of the 8 cores owns 1024 output
columns (weight/weight_scale/bias rows); x replicated.

v8 design (schedule-driven rebuild of v7; measured rates):
  - DVE TT mult caps at 2x (no 4x mode) -> the 8.4M-elem weight-scale
    multiply is a hard ~35us DVE chain; DMA writes ~14MB at ~380GB/s and
    ACT casts are co-critical. v8 minimizes fill/drain latency around that.
  - Head start: first transfers (wsT c0, g0 chunks) issue on the SYNC (SP)
    HWDGE queue whose framework preamble retires ~2.5us before GpSimd's
    SWDGE queue. g0 is consumed chunk-by-chunk as data lands.
  - Steady state: half-group granularity everywhere (TT [128,2048] and
    half-group ACT casts) so every stage is consumable ~1-2us after its
    bytes land; delivery order strictly matches consumption order on one
    SWDGE ring. 7 groups land fp8 + ACT-cast (halves); 8 land via
    cast-DMA bf16; head group chunk-granular cast-DMA.
  - x-side dequant sits in the fill gap between the head group and the
    first fp8 group's cast.
  - Tail: psum written as two 512-col halves; q0 copy/DMA overlaps q1's
    final matmuls; outputs leave on the idle SYNC/ACT HWDGE queues.

Host-side marshaling stays format-only (LUT decode + layout + dtype cast);
all reference arithmetic (scale multiplies, matmul, bias) is on device.
"""

import json
from contextlib import ExitStack

import ml_dtypes
import numpy as np

import concourse.bass as bass
import concourse.mybir as mybir
import concourse.tile as tile
from concourse.bass_utils import run_bass_kernel_spmd


def _split_multi_waits(m: dict) -> dict:
    """This walrus build allows at most one sync-wait command per instruction.
    Hoist extra waits into standalone EventSemaphore instructions issued just
    before the owning instruction on the same engine queue (semantically
    identical: the engine stalls in order)."""
    for fn in m["functions"]:
        for blk in fn["blocks"]:
            new = []
            ctr = 0
            for inst in blk["instructions"]:
                si = inst.get("sync_info")
                waits = (si or {}).get("on_wait") or []
                if len(waits) > 1:
                    for w in waits[:-1]:
                        new.append({
                            "debug": inst.get("debug", 0),
                            "engine": inst["engine"],
                            "ins": [],
                            "outs": [],
                            "name": f"{inst['name']}-hw{ctr}",
                            "opcode": "EventSemaphore",
                            "sync_info": {"on_update": [], "on_wait": [w]},
                        })
                        ctr += 1
                    si["on_wait"] = [waits[-1]]
                new.append(inst)
            blk["instructions"] = new
    return m


class _SplitWaitBass(bass.Bass):
    def to_json_bytes(self) -> bytes:
        m = json.loads(super().to_json_bytes())
        return json.dumps(_split_multi_waits(m)).encode()


BF16 = ml_dtypes.bfloat16
FP8 = ml_dtypes.float8_e4m3
FP4_LUT = np.array(
    [0.0, 0.5, 1.0, 1.5, 2.0, 3.0, 4.0, 6.0,
     -0.0, -0.5, -1.0, -1.5, -2.0, -3.0, -4.0, -6.0],
    dtype=np.float32,
)

M, K, N = 64, 8192, 8192
NCORES = 8
NS = N // NCORES        # 1024 output columns per core
BLOCK = 16
B = K // BLOCK          # 512 scale blocks along K
P = 128                 # partitions
CHUNKS = K // P         # 64 K-chunks
CB = B // P             # 4 scale-chunk columns (c index)
J = BLOCK               # 16 j-groups (one group = CB chunks = 512 rows)
GW = CB * NS            # 4096 columns per weight group tile
NQ = NS                 # 1024 columns per chunk of a group tile
HQ = 2 * NQ             # 2048 columns per half-group (u index)

# Slot types, g = 0..15 in delivery order (== j index; identity layout).
# 'C' chunk-granular head (c0/c1 bf16-direct on sync, c2/c3 cast-DMA),
# 'S' cast-DMA bf16, 'A' fp8 + ACT half-cast.
TYPES = "CASASASASASASAAS"

_CACHE: dict = {}


def _build_program() -> bass.Bass:
    nc = _SplitWaitBass("TRN2", target_bir_lowering=False, debug=False,
                        num_devices=NCORES)
    dt = mybir.dt

    wvp = nc.dram_tensor("wvp", [P, J * GW], dt.float8e4,
                         kind="ExternalInput").ap()
    wv0b = nc.dram_tensor("wv0b", [P, GW], dt.bfloat16,
                          kind="ExternalInput").ap()
    wst = nc.dram_tensor("wst", [P, GW], dt.bfloat16,
                         kind="ExternalInput").ap()
    xvp = nc.dram_tensor("xvp", [P, CHUNKS * M], dt.bfloat16,
                         kind="ExternalInput").ap()
    xst = nc.dram_tensor("xst", [P, CB * M], dt.bfloat16,
                         kind="ExternalInput").ap()
    gs = nc.dram_tensor("gs", [P, 2], dt.float32, kind="ExternalInput").ap()
    bia = nc.dram_tensor("bia", [1, NS], dt.bfloat16, kind="ExternalInput").ap()
    out = nc.dram_tensor("out", [M, NS], dt.bfloat16, kind="ExternalOutput").ap()

    def wv_slab(g):
        return wvp[:, g * GW:(g + 1) * GW]

    with tile.TileContext(nc) as tc, ExitStack() as ctx:
        const = ctx.enter_context(tc.tile_pool(name="const", bufs=1))
        w8pool = ctx.enter_context(tc.tile_pool(name="w8", bufs=1))
        whpool = ctx.enter_context(tc.tile_pool(name="wh", bufs=1))
        ppool = ctx.enter_context(tc.tile_pool(name="acc", bufs=1, space="PSUM"))

        wsT = const.tile([P, GW], dt.bfloat16)
        xva = const.tile([P, CHUNKS * M], dt.bfloat16)
        gt = const.tile([P, 2], dt.float32)
        xsT = const.tile([P, CB * M], dt.bfloat16)
        bsb = const.tile([1, NS], dt.bfloat16)
        ones = const.tile([1, M], dt.bfloat16)
        gcol = const.tile([P, 1], dt.float32)
        xsb = const.tile([P, CB * M], dt.bfloat16)
        xhat = const.tile([P, CHUNKS * M], dt.bfloat16)
        psum = ppool.tile([M, NS], dt.float32)
        osb = const.tile([M, NS], dt.bfloat16)

        wbf: dict = {}       # bf16 weight tiles per group
        w8t: dict = {}       # fp8 slabs for A groups (dedicated)
        for g, ty in enumerate(TYPES):
            wbf[g] = whpool.tile([P, GW], dt.bfloat16, name=f"wbf{g}")
            if ty == "A":
                w8t[g] = w8pool.tile([P, GW], dt.float8e4, name=f"w8_{g}")

        def cs(c):
            return slice(c * NQ, (c + 1) * NQ)

        def us(u):
            return slice(u * HQ, (u + 1) * HQ)

        # ================= DMA delivery =================
        # SYNC (SP) HWDGE queue frees earliest (no cast support): g0 comes
        # straight from a bf16 copy in DRAM, chunk by chunk.
        nc.sync.dma_start(wbf[0][:, cs(0)], wv0b[:, cs(0)])
        nc.sync.dma_start(wsT[:, us(0)], wst[:, us(0)])
        nc.sync.dma_start(wbf[0][:, cs(1)], wv0b[:, cs(1)])
        nc.sync.dma_start(wbf[0][:, cs(2)], wv0b[:, cs(2)])
        nc.sync.dma_start(wbf[0][:, cs(3)], wv0b[:, cs(3)])

        # SWDGE ring (gpsimd): strict consumption order; A slabs run one
        # slot ahead of their TT slot to hide the ACT cast latency.
        nc.gpsimd.dma_start(wsT[:, us(1)], wst[:, us(1)])
        nc.gpsimd.dma_start(gt[:], gs[:])
        nc.gpsimd.dma_start(xsT[:], xst[:])
        nc.gpsimd.dma_start(bsb[:], bia[:])
        nc.gpsimd.dma_start(w8t[1][:], wv_slab(1))
        nc.gpsimd.dma_start(xva[:, us(0)], xvp[:, us(0)])
        nc.gpsimd.dma_start(xva[:, us(1)], xvp[:, us(1)])
        for g in range(2, J):
            # deliver each A slab before the S of the preceding slot
            if TYPES[g] == "A":
                continue
            nxt = g + 1
            if nxt < J and TYPES[nxt] == "A":
                nc.gpsimd.dma_start(w8t[nxt][:], wv_slab(nxt))
            if nxt + 1 < J and TYPES[nxt + 1] == "A":
                nc.gpsimd.dma_start(w8t[nxt + 1][:], wv_slab(nxt + 1))
            nc.gpsimd.dma_start(wbf[g][:], wv_slab(g))

        # ================= ACT cast chain (halves) =================
        for g in range(1, J):
            if TYPES[g] == "A":
                for u in range(2):
                    nc.scalar.copy(wbf[g][:, us(u)], w8t[g][:, us(u)])

        # ================= matmul emission helper =================
        def emit_mms(g, c, q, stop=False):
            t = g * CB + c
            nc.tensor.matmul(
                psum[:, q * 512:(q + 1) * 512],
                xhat[:, t * M:(t + 1) * M],
                wbf[g][:, c * NQ + q * 512: c * NQ + (q + 1) * 512],
                start=False,
                stop=stop,
            )

        # ================= DVE chain + PE, in slot order =================
        nc.vector.memset(ones[:], 1.0)

        # head group g0: chunk-granular TTs, x-scale prep in the c1->c2 gap
        for c in range(CB):
            nc.vector.tensor_mul(wbf[0][:, cs(c)], wbf[0][:, cs(c)],
                                 wsT[:, cs(c)])
            if c == 1:
                nc.vector.tensor_mul(gcol[:], gt[:, 0:1], gt[:, 1:2])
                nc.vector.tensor_scalar_mul(xsb[:], xsT[:], gcol[:])

        # x-side dequant, in two j-halves (fills the gap before the first
        # A cast completes; half u covers the chunks of groups 8u..8u+7)
        xsb_b = xsb[:].unsqueeze(1).broadcast_to([P, J // 2, CB * M])
        for u in range(2):
            nc.vector.tensor_mul(
                xhat[:, us(u)].rearrange("p (j w) -> p j w", j=J // 2),
                xva[:, us(u)].rearrange("p (j w) -> p j w", j=J // 2),
                xsb_b,
            )

        # bias init via ones-matmul, then g0 matmuls
        for q in range(2):
            nc.tensor.matmul(
                psum[:, q * 512:(q + 1) * 512],
                ones[:1, :],
                bsb[:1, q * 512:(q + 1) * 512],
                start=True,
                stop=False,
            )
        for c in range(CB):
            for q in range(2):
                emit_mms(0, c, q)

        # steady state: half-group TTs + matmuls per half
        for g in range(1, J):
            last_g = g == J - 1
            for u in range(2):
                nc.vector.tensor_mul(wbf[g][:, us(u)], wbf[g][:, us(u)],
                                     wsT[:, us(u)])
                if not last_g:
                    for c in (2 * u, 2 * u + 1):
                        for q in range(2):
                            emit_mms(g, c, q)
            # tail group: q0 matmuls first so the q0 copy/DMA overlaps q1
        g = J - 1
        for q in range(2):
            for c in range(CB):
                emit_mms(g, c, q, stop=(c == CB - 1))
            if q == 0:
                nc.vector.tensor_copy(osb[:, 0:512], psum[:, 0:512])
                nc.sync.dma_start(out[:, 0:512], osb[:, 0:512])
        nc.scalar.copy(osb[:, 512:NS], psum[:, 512:NS])
        nc.scalar.dma_start(out[:, 512:NS], osb[:, 512:NS])

    return nc


def _perm_k(vals_2d: np.ndarray) -> np.ndarray:
    """[R, K] fp values -> [K, R] with K permuted as r = j*B + b."""
    r = vals_2d.shape[0]
    return (
        vals_2d.reshape(r, B, BLOCK).transpose(2, 1, 0).reshape(K, r)
    )


def _swz(rows_2d: np.ndarray, width: int) -> np.ndarray:
    """[n_chunks*128, width] -> [128, n_chunks*width]: row p holds chunk-major
    data for partition p (per-partition-contiguous DMA layout)."""
    n = rows_2d.shape[0] // P
    return np.ascontiguousarray(
        rows_2d.reshape(n, P, width).transpose(1, 0, 2).reshape(P, n * width)
    )


def prepare_in_maps(**inputs) -> list[dict[str, np.ndarray]]:
    x = np.asarray(inputs["x"]).astype(np.int64)
    xs = np.asarray(inputs["x_scale"], dtype=np.float32)
    gx = np.float32(np.asarray(inputs["x_global_scale"]).reshape(-1)[0])
    w = np.asarray(inputs["weight"]).astype(np.int64)
    ws = np.asarray(inputs["weight_scale"], dtype=np.float32)
    gw = np.float32(np.asarray(inputs["weight_global_scale"]).reshape(-1)[0])
    b = np.asarray(inputs["bias"], dtype=np.float32)

    xvp = _swz(_perm_k(FP4_LUT[x]).astype(BF16), M)                  # [128, 4096]
    xst = _swz(np.ascontiguousarray(xs.T), M).astype(BF16)           # [128, 256]
    gsv = np.tile(np.array([[gx, gw]], dtype=np.float32), (P, 1))

    wv = FP4_LUT[w]                                                  # [N, K] f32
    in_maps = []
    for c in range(NCORES):
        sl = slice(c * NS, (c + 1) * NS)
        wvp = _swz(_perm_k(wv[sl]).astype(FP8), NS)                  # [128, 64*NS]
        wv0b = np.ascontiguousarray(wvp[:, 0:GW]).astype(BF16)      # g0 bf16
        in_maps.append({
            "wvp": wvp,
            "wv0b": wv0b,
            "wst": _swz(ws[sl].T.astype(BF16), NS),                  # [128, 4*NS]
            "xvp": xvp,
            "xst": xst,
            "gs": gsv,
            "bia": np.ascontiguousarray(b[sl].reshape(1, NS)).astype(BF16),
        })
    return in_maps


LAST_RESULTS = None


def kernel(**inputs) -> np.ndarray:
    global LAST_RESULTS
    if "nc" not in _CACHE:
        _CACHE["nc"] = _build_program()
    nc = _CACHE["nc"]

    in_maps = prepare_in_maps(**inputs)
    res = run_bass_kernel_spmd(nc, in_maps, core_ids=list(range(NCORES)))
    LAST_RESULTS = res
    out = np.concatenate([res.results[c]["out"] for c in range(NCORES)], axis=1)
    return out.astype(BF16)
